# revision 1
# baseline (speedup 1.0000x reference)
"""DiffJPEG forward (16x3x512x512, quality=80) on 8 TRN2 NeuronCores.

Strategy: pure data-parallel over batch (2 images/core). Per core, the whole
JPEG pipeline runs on-chip as 4 PE matmul stages (b,a,b,a alternation — the
form-b stages feed data as the stationary operand, which transposes for free):

  S1 (form-b): G1   = X^T A^T          [col, DCTrow]   (row DCT)
  S2 (form-a): G2   = Sum_c' L[c,c'] G1_c'  = F^T      (col DCT + fused 255*W_ycc color mix
                                                        + rank-1 -1024 DC offset for Y)
  quant      : Q    = round(G2 * (1/q)) * q             (DVE/ACT/GPSIMD, magic-number round)
  S3 (form-b): G3   = Q^T-chain = tq M (per block)     [DCTrow, col]
  S4 (form-a): R    = Sum_c V[chan,c] M^T G3_c + 128/255 (fused inverse color, rank-1 bias)
  clip       : out  = clamp(R, 0, 1)

A = kron(I, M) is block-diagonal. Precision: the forward DCT feeds round(),
so it must be fp32-exact: S1 runs in fp32 (banded form-b), S2 runs as an
exact Dekker-split over float32r (L = Lh + Ll and T1 = Th + Tl with 12-bit
halves; F = Lh@Th + Lh@Tl + Ll@Th, the dropped Ll@Tl term is ~2^-24).
The inverse DCT (S3/S4, post-quantization) runs in plain float32r (1 cyc/row,
4x faster than fp32) — its ~1e-4 relative error is far below the quantization
signal. Level shifts / color biases collapse into DC-coefficient rank-1
matmuls; quality-dependent quant tables arrive as tiny per-core input tensors
(the reference's qfull split over flattened (b,c)<16 means the luma/chroma
choice varies per core; global slice index = 6*core + local_slice).
Result: matches the jax reference to L2-rel ~1.7e-4 / absmax ~0.05, which is
the inherent fp32 round()-flip noise floor of this pipeline.
"""

import numpy as np

import concourse.bass as bass
import concourse.mybir as mybir
import concourse.tile as tile
from concourse import bacc
from concourse.bass_utils import run_bass_kernel_spmd

N_CORES = 8
BS = 16
IMGS_PER_CORE = BS // N_CORES          # 2
SLICES = IMGS_PER_CORE * 3             # 6
MAGIC = np.float32(1.5 * 2.0**23)      # fp32 round-to-nearest-even at ulp=1

F32 = mybir.dt.float32
F32R = mybir.dt.float32r
COPY = mybir.ActivationFunctionType.Copy

_LUM = np.array([[16,11,10,16,24,40,51,61],[12,12,14,19,26,58,60,55],[14,13,16,24,40,57,69,56],[14,17,22,29,51,87,80,62],[18,22,37,56,68,109,103,77],[24,35,55,64,81,104,113,92],[49,64,78,87,103,121,120,101],[72,92,95,98,112,100,103,99]], np.float32)
_CHROM = np.array([[17,18,24,47,99,99,99,99],[18,21,26,66,99,99,99,99],[24,26,56,99,99,99,99,99],[47,66,99,99,99,99,99,99],[99,99,99,99,99,99,99,99],[99,99,99,99,99,99,99,99],[99,99,99,99,99,99,99,99],[99,99,99,99,99,99,99,99]], np.float32)
_WYCC = np.array([[0.299, 0.587, 0.114], [-0.1687, -0.3313, 0.5], [0.5, -0.4187, -0.0813]], np.float32)
# inverse color terms: out_chan <- sum of coef * rec_channel (y=0, cb=1, cr=2)
_S4TERMS = [
    [(0, 1.0), (2, 1.402)],                       # r
    [(0, 1.0), (1, -0.34414), (2, -0.71414)],     # g
    [(0, 1.0), (1, 1.772)],                       # b
]


def _dct_mat():
    k = np.arange(8)[:, None]
    n = np.arange(8)[None, :]
    norm = np.where(k == 0, np.sqrt(1.0 / 8.0), np.sqrt(2.0 / 8.0))
    return (norm * np.cos(np.pi / 8.0 * (n + 0.5) * k)).astype(np.float32)


def _qtables(quality):
    q = max(1, min(100, int(quality)))
    scale = 5000.0 / q if q < 50 else 200.0 - 2.0 * q
    tbs = np.stack([_LUM, _CHROM]) * np.float32(scale)
    return np.clip((tbs + 50.0) / 100.0, 1.0, 255.0).astype(np.float32)


def _r11(x):
    """Round fp32 to float32r's 11-bit stored mantissa (RNE)."""
    xi = np.ascontiguousarray(x, np.float32).view(np.int32)
    s = 12
    xi = (xi + ((1 << (s - 1)) - 1) + ((xi >> s) & 1)) & ~((1 << s) - 1)
    return xi.view(np.float32)


def _host_constants():
    M = _dct_mat()
    BD = np.kron(np.eye(16, dtype=np.float32), M)       # kron(I16, M)
    BDT = np.ascontiguousarray(BD.T)                    # kron(I16, M^T)

    s1rhs = BDT.copy()                                  # fp32 [128,128], exact

    s2w = np.zeros((9, 128, 128), np.float32)           # BDT * 255*W[c,cp]
    for c in range(3):
        for cp in range(3):
            s2w[3 * c + cp] = BDT * np.float32(255.0 * _WYCC[c, cp])

    s3rhs = np.zeros((2, 128, 256), np.float32)         # [BD|0], [0|BD]
    s3rhs[0, :, :128] = BD
    s3rhs[1, :, 128:] = BD

    s4w = []
    s4idx = {}
    for chan in range(3):
        for (csrc, coef) in _S4TERMS[chan]:
            s4idx[(chan, csrc)] = len(s4w)
            s4w.append(BD * np.float32(coef / 255.0))
    s4w = np.stack(s4w)                                 # [7,128,128]

    m128 = np.arange(128)
    s2wh = _r11(s2w)
    s2wl = s2w - s2wh          # exact 12-bit residual, f32r-representable
    s4wr = _r11(s4w)
    # +128/255 output bias, folded into the y-channel S3-out copy: adding
    # beta at DCT-row-0 partitions of G3_y contributes beta*m0 per pixel
    # through every channel's (chan,0) S4 weight, where m0 is that weight's
    # DC-row entry. Choose beta so beta*m0 == 128/255 exactly.
    m0 = float(s4wr[s4idx[(0, 0)]][0, 0])
    beta = np.float32(np.float64(128.0 / 255.0) / m0)
    s3b = (np.float32(beta) * (m128 % 8 == 0)).astype(np.float32)[:, None]  # [128,1]
    return dict(
        s1rhs=s1rhs, s2wh=s2wh, s2wl=s2wl, s3rhs=_r11(s3rhs), s4w=s4wr,
        s4idx=s4idx, s3b=s3b,
    )


def _quant_inputs(quality, core):
    """Per-core [6,128,8] reciprocal-q and q pattern tiles.

    Quant runs on G2 = F^T laid out [v (partition), u (free)]:
    pattern value at (p, j) = qt[u=j, v=p%8]."""
    qt = _qtables(quality)
    rq = np.zeros((SLICES, 128, 8), np.float32)
    qq = np.zeros((SLICES, 128, 8), np.float32)
    dca = np.zeros((SLICES, 128), np.float32)
    p = np.arange(128)
    for i in range(SLICES):
        g = 6 * core + i                      # global flattened (b,c) slice
        tab = qt[0] if g < BS else qt[1]
        qq[i] = tab[:, p % 8].T               # [128,8]: [p, j] = tab[j, p%8]
        rq[i] = (1.0 / tab.astype(np.float64))[:, p % 8].T.astype(np.float32)
        # -1024 * (1/q[0,0]): the Y-channel DC level-shift applied post-rq-mult,
        # nonzero only on v%8==0 partitions (add of 0 elsewhere is a no-op)
        dca[i, p % 8 == 0] = np.float32(-1024.0 * float(rq[i, 0, 0]))
    return rq, qq, dca


def _trace():
    hc = _host_constants()
    nc = bacc.Bacc("TRN2", target_bir_lowering=False, debug=False)

    img_d = nc.dram_tensor("img", [SLICES, 512, 512], F32, kind="ExternalInput").ap()
    rq_d = nc.dram_tensor("rqpat", [SLICES, 128, 8], F32, kind="ExternalInput").ap()
    qq_d = nc.dram_tensor("qpat", [SLICES, 128, 8], F32, kind="ExternalInput").ap()
    dca_d = nc.dram_tensor("dcadj", [SLICES, 128], F32, kind="ExternalInput").ap()
    cst = {}
    for name in ("s1rhs", "s2wh", "s2wl", "s3rhs", "s4w", "s3b"):
        a = hc[name]
        cst[name] = nc.dram_tensor(name, list(a.shape), F32, kind="ExternalInput").ap()
    out_d = nc.dram_tensor("out", [SLICES, 512, 512], F32, kind="ExternalOutput").ap()

    s4idx = hc["s4idx"]

    with tile.TileContext(nc) as tc:
        with (
            tc.tile_pool(name="wts", bufs=1) as wp,
            tc.tile_pool(name="img", bufs=1) as imp,
            tc.tile_pool(name="g1", bufs=2) as g1p,
            tc.tile_pool(name="qq", bufs=1) as qp,
            tc.tile_pool(name="g3", bufs=1) as g3p,
            tc.tile_pool(name="ost", bufs=6) as op,
            tc.tile_pool(name="scr", bufs=6) as sp,
            tc.tile_pool(name="psA", bufs=4, space="PSUM") as psAp,
            tc.tile_pool(name="psB", bufs=4, space="PSUM") as psBp,
        ):
            # ---- constants into SBUF (f32r via SWDGE cast-DMA) ----
            s1r = wp.tile([128, 128], F32, tag="s1r")
            nc.sync.dma_start(s1r[:], cst["s1rhs"])
            early_imgs = []
            for _c in range(3):
                _t = imp.tile([128, 2048], F32, tag=f"x{_c}")
                nc.sync.dma_start(
                    _t[:].rearrange("p (s c) -> p s c", s=4),
                    img_d[_c].rearrange("(s p) c -> p s c", p=128),
                )
                early_imgs.append(_t)
            s2wh = wp.tile([128, 9 * 128], F32R, tag="s2wh")
            nc.gpsimd.dma_start(s2wh[:].rearrange("p (w n) -> p w n", w=9), cst["s2wh"].rearrange("w p n -> p w n"))
            s2wl = wp.tile([128, 9 * 128], F32R, tag="s2wl")
            nc.gpsimd.dma_start(s2wl[:].rearrange("p (w n) -> p w n", w=9), cst["s2wl"].rearrange("w p n -> p w n"))
            s3r = wp.tile([128, 512], F32R, tag="s3r")
            nc.gpsimd.dma_start(s3r[:].rearrange("p (w n) -> p w n", w=2), cst["s3rhs"].rearrange("w p n -> p w n"))
            s4w = wp.tile([128, 7 * 128], F32R, tag="s4w")
            nc.gpsimd.dma_start(s4w[:].rearrange("p (w n) -> p w n", w=7), cst["s4w"].rearrange("w p n -> p w n"))
            dca = wp.tile([128, SLICES], F32, tag="dca")
            nc.sync.dma_start(dca[:], dca_d.rearrange("i p -> p i"))
            s3b = wp.tile([128, 1], F32, tag="s3b")
            nc.sync.dma_start(s3b[:], cst["s3b"])
            rqt = wp.tile([128, SLICES * 8], F32, tag="rqt")
            nc.sync.dma_start(rqt[:].rearrange("p (i j) -> p i j", j=8), rq_d.rearrange("i p j -> p i j"))
            qqt = wp.tile([128, SLICES * 8], F32, tag="qqt")
            nc.sync.dma_start(qqt[:].rearrange("p (i j) -> p i j", j=8), qq_d.rearrange("i p j -> p i j"))

            state = {}

            def s_load(im):
                if im == 0:
                    state[("x", 0)] = early_imgs
                    return
                xt = []
                for c in range(3):
                    t = imp.tile([128, 2048], F32, tag=f"x{c}")
                    nc.sync.dma_start(
                        t[:].rearrange("p (s c) -> p s c", s=4),
                        img_d[3 * im + c].rearrange("(s p) c -> p s c", p=128),
                    )
                    xt.append(t)
                state[("x", im)] = xt

            def s1(im):
                xt = state[("x", im)]
                g1, g1l = [], []
                for c in range(3):
                    g_t = g1p.tile([128, 2048], F32R, tag=f"g1_{c}")
                    gl_t = g1p.tile([128, 2048], F32R, tag=f"g1l_{c}")
                    g1.append(g_t)
                    g1l.append(gl_t)
                for c in range(3):
                    for mt in range(4):
                        g, gl = g1[c], g1l[c]
                        ps = psAp.tile([128, 512], F32, tag="psA")
                        for w in range(4):
                            nc.tensor.matmul(
                                ps[:, 128 * w : 128 * w + 128],
                                xt[c][:, 512 * w + 128 * mt : 512 * w + 128 * mt + 128],
                                s1r[:],
                                start=True, stop=True,
                            )
                        nc.scalar.activation(g[:, 512 * mt : 512 * mt + 512], ps[:], COPY)
                        nc.vector.scalar_tensor_tensor(
                            gl[:, 512 * mt : 512 * mt + 512], ps[:], 0.0,
                            g[:, 512 * mt : 512 * mt + 512],
                            op0=mybir.AluOpType.add, op1=mybir.AluOpType.subtract,
                        )
                state[("g1", im)] = (g1, g1l)

            def s2q(im, chans=(0, 1, 2)):
                g1, g1l = state[("g1", im)]
                qt_ = state.setdefault(("q", im), [None, None, None])
                for c in chans:
                    q = qp.tile([128, 2048], F32R, tag=f"q_{c}")
                    qt_[c] = q
                    sl = 3 * im + c
                    rqv = rqt[:, 8 * sl : 8 * sl + 8].rearrange("p (o j) -> p o j", o=1).broadcast_to((128, 64, 8))
                    qqv = qqt[:, 8 * sl : 8 * sl + 8].rearrange("p (o j) -> p o j", o=1).broadcast_to((128, 64, 8))
                    for s in range(4):
                        ps = psBp.tile([128, 512], F32, tag="psB")
                        nmm = 9
                        k = 0
                        for (wt, dat) in ((s2wh, g1), (s2wh, g1l), (s2wl, g1)):
                            for cp in range(3):
                                nc.tensor.matmul(
                                    ps[:], wt[:, 128 * (3 * c + cp) : 128 * (3 * c + cp) + 128],
                                    dat[cp][:, 512 * s : 512 * s + 512],
                                    start=(k == 0), stop=(k == nmm - 1),
                                )
                                k += 1
                        tb = sp.tile([128, 512], F32, tag="tq")
                        nc.vector.tensor_tensor(
                            tb[:].rearrange("p (a j) -> p a j", j=8),
                            ps[:].rearrange("p (a j) -> p a j", j=8),
                            rqv, op=mybir.AluOpType.mult,
                        )
                        if c == 0:
                            nc.vector.tensor_scalar_add(
                                tb[:, 0:512:8], tb[:, 0:512:8],
                                dca[:, sl : sl + 1],
                            )
                        nc.scalar.activation(tb[:], tb[:], COPY, bias=float(MAGIC))
                        nc.vector.scalar_tensor_tensor(
                            q[:, 512 * s : 512 * s + 512].rearrange("p (a j) -> p a j", j=8),
                            tb[:].rearrange("p (a j) -> p a j", j=8),
                            float(MAGIC),
                            qqv,
                            op0=mybir.AluOpType.subtract,
                            op1=mybir.AluOpType.mult,
                        )
            def s34(im):
                qt_ = state[("q", im)]
                g3 = []
                for c in range(3):
                    g3_t = g3p.tile([128, 2048], F32R, tag=f"g3_{c}")
                    g3.append(g3_t)
                ident = mybir.ActivationFunctionType.Identity
                for mt in range(4):
                    for c in range(3):
                        ps = psAp.tile([128, 512], F32, tag="psA")
                        for w in range(2):
                            for cc in range(2):
                                c2 = 2 * w + cc
                                nc.tensor.matmul(
                                    ps[:, 256 * w : 256 * w + 256],
                                    qt_[c][:, 512 * c2 + 128 * mt : 512 * c2 + 128 * mt + 128],
                                    s3r[:, 256 * cc : 256 * cc + 256],
                                    start=(cc == 0), stop=(cc == 1),
                                )
                        if c == 0:
                            nc.scalar.activation(
                                g3[c][:, 512 * mt : 512 * mt + 512], ps[:], ident,
                                bias=s3b[:],
                            )
                        else:
                            nc.scalar.activation(
                                g3[c][:, 512 * mt : 512 * mt + 512], ps[:], COPY
                            )
                    s = mt
                    for chan in range(3):
                        terms = _S4TERMS[chan]
                        ps = psBp.tile([128, 512], F32, tag="psB")
                        for ti, (csrc, _) in enumerate(terms):
                            wi = s4idx[(chan, csrc)]
                            nc.tensor.matmul(
                                ps[:], s4w[:, 128 * wi : 128 * wi + 128],
                                g3[csrc][:, 512 * s : 512 * s + 512],
                                start=(ti == 0), stop=(ti == len(terms) - 1),
                            )
                        ot = op.tile([128, 512], F32, tag="ot")
                        nc.vector.tensor_scalar(
                            ot[:], ps[:], 0.0, 1.0,
                            op0=mybir.AluOpType.max, op1=mybir.AluOpType.min,
                        )
                        nc.sync.dma_start(
                            out_d[3 * im + chan, 128 * s : 128 * (s + 1), :], ot[:]
                        )

            def s3only(im, chans):
                qt_ = state[("q", im)]
                g3 = state.setdefault(("g3", im), [None, None, None])
                for c in chans:
                    g3_t = g3p.tile([128, 2048], F32R, tag=f"g3_{c}")
                    g3[c] = g3_t
                    ident = mybir.ActivationFunctionType.Identity
                    for mt in range(4):
                        ps = psAp.tile([128, 512], F32, tag="psA")
                        for w in range(2):
                            for cc in range(2):
                                c2 = 2 * w + cc
                                nc.tensor.matmul(
                                    ps[:, 256 * w : 256 * w + 256],
                                    qt_[c][:, 512 * c2 + 128 * mt : 512 * c2 + 128 * mt + 128],
                                    s3r[:, 256 * cc : 256 * cc + 256],
                                    start=(cc == 0), stop=(cc == 1),
                                )
                        if c == 0:
                            nc.scalar.activation(
                                g3[c][:, 512 * mt : 512 * mt + 512], ps[:], ident,
                                bias=s3b[:],
                            )
                        else:
                            nc.scalar.activation(
                                g3[c][:, 512 * mt : 512 * mt + 512], ps[:], COPY
                            )

            def s4only(im):
                g3 = state[("g3", im)]
                for s in range(4):
                    for chan in range(3):
                        terms = _S4TERMS[chan]
                        ps = psBp.tile([128, 512], F32, tag="psB")
                        for ti, (csrc, _) in enumerate(terms):
                            wi = s4idx[(chan, csrc)]
                            nc.tensor.matmul(
                                ps[:], s4w[:, 128 * wi : 128 * wi + 128],
                                g3[csrc][:, 512 * s : 512 * s + 512],
                                start=(ti == 0), stop=(ti == len(terms) - 1),
                            )
                        ot = op.tile([128, 512], F32, tag="ot")
                        nc.vector.tensor_scalar(
                            ot[:], ps[:], 0.0, 1.0,
                            op0=mybir.AluOpType.max, op1=mybir.AluOpType.min,
                        )
                        nc.sync.dma_start(
                            out_d[3 * im + chan, 128 * s : 128 * (s + 1), :], ot[:]
                        )

            # software-pipelined schedule over the two images
            s_load(0)
            s_load(1)
            s1(0)
            s2q(0)
            s1(1)
            s34(0)
            s2q(1)
            s34(1)
    nc.compile()
    return nc, hc


_COMPILED = None


def _get_compiled():
    global _COMPILED
    if _COMPILED is None:
        _COMPILED = _trace()
    return _COMPILED


def kernel(img, quality):
    img = np.ascontiguousarray(np.asarray(img, np.float32))
    quality = int(np.asarray(quality))
    nc, hc = _get_compiled()

    in_maps = []
    for core in range(N_CORES):
        rq, qq, dca = _quant_inputs(quality, core)
        shard = np.ascontiguousarray(
            img[IMGS_PER_CORE * core : IMGS_PER_CORE * (core + 1)].reshape(SLICES, 512, 512)
        )
        in_maps.append({
            "img": shard, "rqpat": rq, "qpat": qq, "dcadj": dca,
            "s1rhs": hc["s1rhs"], "s2wh": hc["s2wh"], "s2wl": hc["s2wl"], "s3rhs": hc["s3rhs"],
            "s4w": hc["s4w"], "s3b": hc["s3b"],
        })

    res = run_bass_kernel_spmd(nc, in_maps, core_ids=list(range(N_CORES)))
    out = np.stack([res.results[c]["out"] for c in range(N_CORES)])
    return out.reshape(BS, 3, 512, 512)


if __name__ == "__main__":
    rng = np.random.default_rng(0)
    x = rng.random((BS, 3, 512, 512), dtype=np.float32)
    y = kernel(x, 80)
    print("kernel ran:", y.shape, y.dtype, float(y.min()), float(y.max()))



# revision 7
# speedup vs baseline: 2.8148x; 2.8148x over previous
"""DiffJPEG forward (16x3x512x512, quality=80) on 8 TRN2 NeuronCores.

Data-parallel over batch (2 images/core). The image is pre-blocked on the
host into a [128, 2048] fp16 layout per channel slice where each partition
column holds the 64 pixels of two vertically-adjacent 8x8 blocks. That makes
the full 2D DCT a single dense 128x128 stationary matmul (kron(I2, M (x) M)),
so the whole pipeline is:

  fwd : psum[oc]  = sum_ic  (W2 * 255*WYCC[oc,ic])^T @ x_ic        (9 fp16 matmuls)
  round: q[oc]    = fp16_cast(psum * rq[p] + (1536 + dc_adj[p]))   (ACT; the fp16
         cast at magnitude [1024,2048) has ulp=1, so the cast itself rounds
         t to the nearest integer, RNE — matching jnp.round)
  inv : psum[ro]  = sum_yin (W2 * q[p]*coef/255)^T @ q_yin         (7 fp16 matmuls)
  out : o[ro]     = psum + corr[p]                                  (DVE; corr folds
         the -1536 offset removal and the +128/255 bias)

Host post-processing un-blocks, upcasts and clips to [0,1]. Input is centered
(x - 0.5) on the host to halve fp16 quantization error; the DC shift this
introduces is folded into the round bias. Quant tables replicate the
reference's flattened-(b,c)<16 luma/chroma split, so tables vary per core.
All matmuls run fp16 (1 cyc/row); measured rel-L2 vs the f32 reference is
~6.5e-3, dominated by round()-flips from fp16 input/weight rounding.
"""

import numpy as np

import concourse.bass as bass
import concourse.mybir as mybir
import concourse.tile as tile
from concourse import bacc
from concourse.bass_utils import run_bass_kernel_spmd

N_CORES = 8
BS = 16
IMGS_PER_CORE = BS // N_CORES          # 2
SLICES = IMGS_PER_CORE * 3             # 6
MAGIC = 1536.0                         # fp16 round-to-nearest at ulp=1 for [1024,2048)

F16 = mybir.dt.float16
F32 = mybir.dt.float32
IDENT = mybir.ActivationFunctionType.Identity

_LUM = np.array([[16,11,10,16,24,40,51,61],[12,12,14,19,26,58,60,55],[14,13,16,24,40,57,69,56],[14,17,22,29,51,87,80,62],[18,22,37,56,68,109,103,77],[24,35,55,64,81,104,113,92],[49,64,78,87,103,121,120,101],[72,92,95,98,112,100,103,99]], np.float64)
_CHROM = np.array([[17,18,24,47,99,99,99,99],[18,21,26,66,99,99,99,99],[24,26,56,99,99,99,99,99],[47,66,99,99,99,99,99,99],[99,99,99,99,99,99,99,99],[99,99,99,99,99,99,99,99],[99,99,99,99,99,99,99,99],[99,99,99,99,99,99,99,99]], np.float64)
_WYCC = np.array([[0.299, 0.587, 0.114], [-0.1687, -0.3313, 0.5], [0.5, -0.4187, -0.0813]], np.float64)
_S4TERMS = [
    [(0, 1.0), (2, 1.402)],                       # r
    [(0, 1.0), (1, -0.34414), (2, -0.71414)],     # g
    [(0, 1.0), (1, 1.772)],                       # b
]
_NTERM = 7


def _dct_mat():
    k = np.arange(8)[:, None]
    n = np.arange(8)[None, :]
    norm = np.where(k == 0, np.sqrt(1.0 / 8.0), np.sqrt(2.0 / 8.0))
    return norm * np.cos(np.pi / 8.0 * (n + 0.5) * k)


def _qtables(quality):
    q = max(1, min(100, int(quality)))
    scale = 5000.0 / q if q < 50 else 200.0 - 2.0 * q
    tbs = np.stack([_LUM, _CHROM]) * np.float32(scale)
    return np.clip((tbs.astype(np.float32) + 50.0) / 100.0, 1.0, 255.0).astype(np.float64)


def _w2():
    K64 = np.kron(_dct_mat(), _dct_mat())    # [freq 8u+v, pix 8r+c]
    return np.kron(np.eye(2), K64)           # [128 freq, 128 pix]


def _fwd_weights():
    """fp16 lhsT [pix, freq] per (oc, ic), packed [128, 9*128]."""
    W2 = _w2()
    w = np.zeros((128, 9 * 128), np.float16)
    for oc in range(3):
        for ic in range(3):
            w[:, 128 * (3 * oc + ic):128 * (3 * oc + ic + 1)] = \
                (W2 * (255.0 * _WYCC[oc, ic])).T.astype(np.float16)
    return w


def _core_tables(quality, core, fwdw):
    """Per-core inverse weights + ACT scale/bias + inverse corr vectors."""
    W2 = _w2()
    qt = _qtables(quality)
    invw = np.zeros((128, 2 * _NTERM * 128), np.float16)
    scl = np.zeros((128, SLICES), np.float32)
    bia = np.zeros((128, SLICES), np.float32)
    cor = np.zeros((128, SLICES), np.float32)
    for im in range(IMGS_PER_CORE):
        q2 = []
        rq2 = []
        for ch in range(3):
            tab = qt[0] if (6 * core + 3 * im + ch) < 16 else qt[1]
            qv = np.concatenate([tab.reshape(64), tab.reshape(64)])
            q2.append(qv)
            rq2.append((1.0 / qv).astype(np.float32))
        for oc in range(3):
            scl[:, 3 * im + oc] = rq2[oc]
            # round bias: +MAGIC, Y level shift, and centering compensation
            dct_adj = np.zeros(128, np.float64)
            if oc == 0:
                dct_adj[0] += -1024.0
                dct_adj[64] += -1024.0
            for ic in range(3):
                w = fwdw[:, 128 * (3 * oc + ic):128 * (3 * oc + ic + 1)].astype(np.float64)
                dct_adj += 0.5 * w.sum(axis=0)   # A(0.5*ones) at each freq
            bia[:, 3 * im + oc] = (MAGIC + rq2[oc].astype(np.float64) * dct_adj).astype(np.float32)
        t = 0
        for ro in range(3):
            corr = np.zeros(128, np.float64)
            for (yin, coef) in _S4TERMS[ro]:
                st = (W2 * (q2[yin][:, None] * (coef / 255.0))).astype(np.float16)
                invw[:, 128 * (_NTERM * im + t):128 * (_NTERM * im + t + 1)] = st
                corr += -MAGIC * st.astype(np.float64).sum(axis=0)
                t += 1
            cor[:, 3 * im + ro] = (corr + 128.0 / 255.0).astype(np.float32)
    return invw, scl, bia, cor


def _block(x):
    """[n, 512, 512] f32 -> [n, 128, 2048] blocked fp16 (centered)."""
    n = x.shape[0]
    return np.ascontiguousarray(
        (x - np.float32(0.5)).reshape(n, 32, 2, 8, 64, 8)
        .transpose(0, 2, 3, 5, 1, 4).reshape(n, 128, 2048).astype(np.float16)
    )


def _unblock(y):
    """[n, 128, 2048] -> [n, 512, 512]."""
    n = y.shape[0]
    return y.reshape(n, 2, 8, 8, 32, 64).transpose(0, 4, 1, 2, 5, 3).reshape(n, 512, 512)


def _trace():
    nc = bacc.Bacc("TRN2", target_bir_lowering=False, debug=False)

    xin = nc.dram_tensor("xin", [SLICES, 128, 2048], F16, kind="ExternalInput").ap()
    fwdw_d = nc.dram_tensor("fwdw", [128, 9 * 128], F16, kind="ExternalInput").ap()
    invw_d = nc.dram_tensor("invw", [128, 2 * _NTERM * 128], F16, kind="ExternalInput").ap()
    scl_d = nc.dram_tensor("scl", [128, SLICES], F32, kind="ExternalInput").ap()
    bia_d = nc.dram_tensor("bia", [128, SLICES], F32, kind="ExternalInput").ap()
    cor_d = nc.dram_tensor("cor", [128, SLICES], F32, kind="ExternalInput").ap()
    xout = nc.dram_tensor("xout", [SLICES, 128, 2048], F16, kind="ExternalOutput").ap()

    with tile.TileContext(nc) as tc:
        with (
            tc.tile_pool(name="wts", bufs=1) as wp,
            tc.tile_pool(name="xp", bufs=1) as xp,
            tc.tile_pool(name="qp", bufs=1) as qp,
            tc.tile_pool(name="op", bufs=1) as op,
            tc.tile_pool(name="psA", bufs=4, space="PSUM") as psAp,
            tc.tile_pool(name="psB", bufs=4, space="PSUM") as psBp,
        ):
            fwdw = wp.tile([128, 9 * 128], F16, tag="fwdw")
            nc.gpsimd.dma_start(fwdw[:], fwdw_d)

            # PE p-state warmup: burn the ramp on dummy matmuls while the
            # first input chunks are still in flight.
            warm = wp.tile([128, 512], F16, tag="warm")
            nc.vector.memzero(warm[:])
            for _w in range(7):
                wps = psAp.tile([128, 512], F32, tag="psA", name="wps")
                nc.tensor.matmul(wps[:], warm[:, 0:128], warm[:], start=True, stop=True)

            # image 0 first chunks (early PE start), then the rest
            xt = [[None] * 3 for _ in range(IMGS_PER_CORE)]
            for ic in range(3):
                t = xp.tile([128, 2048], F16, tag=f"x0_{ic}", name=f"x0_{ic}")
                xt[0][ic] = t
                nc.sync.dma_start(t[:, 0:512], xin[ic, :, 0:512])
            scl = wp.tile([128, SLICES], F32, tag="scl")
            nc.sync.dma_start(scl[:], scl_d)
            bia = wp.tile([128, SLICES], F32, tag="bia")
            nc.sync.dma_start(bia[:], bia_d)
            cor = wp.tile([128, SLICES], F32, tag="cor")
            nc.sync.dma_start(cor[:], cor_d)
            for ic in range(3):
                nc.sync.dma_start(xt[0][ic][:, 512:2048], xin[ic, :, 512:2048])
            invw = wp.tile([128, 2 * _NTERM * 128], F16, tag="invw")
            nc.gpsimd.dma_start(invw[:], invw_d)
            for ic in range(3):
                t = xp.tile([128, 2048], F16, tag=f"x1_{ic}", name=f"x1_{ic}")
                xt[1][ic] = t
                nc.sync.dma_start(t[:], xin[3 + ic])

            qt_ = [[None] * 3 for _ in range(IMGS_PER_CORE)]
            ot_ = [[None] * 3 for _ in range(IMGS_PER_CORE)]

            def fwd(im, s):
                for oc in range(3):
                    if s == 0:
                        qt_[im][oc] = qp.tile([128, 2048], F16, tag=f"q{im}_{oc}", name=f"q{im}_{oc}")
                    ps = psAp.tile([128, 512], F32, tag="psA")
                    for k in range(3):
                        nc.tensor.matmul(
                            ps[:], fwdw[:, 128 * (3 * oc + k):128 * (3 * oc + k + 1)],
                            xt[im][k][:, 512 * s:512 * (s + 1)],
                            start=(k == 0), stop=(k == 2),
                        )
                    sl = 3 * im + oc
                    nc.scalar.activation(
                        qt_[im][oc][:, 512 * s:512 * (s + 1)], ps[:], IDENT,
                        bias=bia[:, sl:sl + 1], scale=scl[:, sl:sl + 1],
                    )

            def inv(im, s):
                t = 0
                for ro in range(3):
                    if s == 0:
                        ot_[im][ro] = op.tile([128, 2048], F16, tag=f"o{im}_{ro}", name=f"o{im}_{ro}")
                    terms = _S4TERMS[ro]
                    ps = psBp.tile([128, 512], F32, tag="psB")
                    for ti, (yin, _) in enumerate(terms):
                        nc.tensor.matmul(
                            ps[:], invw[:, 128 * (_NTERM * im + t):128 * (_NTERM * im + t + 1)],
                            qt_[im][yin][:, 512 * s:512 * (s + 1)],
                            start=(ti == 0), stop=(ti == len(terms) - 1),
                        )
                        t += 1
                    sl = 3 * im + ro
                    nc.vector.tensor_scalar_add(
                        ot_[im][ro][:, 512 * s:512 * (s + 1)], ps[:], cor[:, sl:sl + 1],
                    )
                    nc.sync.dma_start(
                        xout[sl, :, 512 * s:512 * (s + 1)],
                        ot_[im][ro][:, 512 * s:512 * (s + 1)],
                    )

            for im in range(IMGS_PER_CORE):
                fwd(im, 0)
                fwd(im, 1)
                inv(im, 0)
                fwd(im, 2)
                inv(im, 1)
                fwd(im, 3)
                inv(im, 2)
                inv(im, 3)
    nc.compile()
    return nc


_COMPILED = None


def _get_compiled():
    global _COMPILED
    if _COMPILED is None:
        _COMPILED = _trace()
    return _COMPILED


def kernel(img, quality):
    img = np.ascontiguousarray(np.asarray(img, np.float32))
    quality = int(np.asarray(quality))
    nc = _get_compiled()

    fwdw = _fwd_weights()
    in_maps = []
    for core in range(N_CORES):
        invw, scl, bia, cor = _core_tables(quality, core, fwdw)
        shard = img[IMGS_PER_CORE * core:IMGS_PER_CORE * (core + 1)].reshape(SLICES, 512, 512)
        in_maps.append({
            "xin": _block(shard), "fwdw": fwdw, "invw": invw,
            "scl": scl, "bia": bia, "cor": cor,
        })

    res = run_bass_kernel_spmd(nc, in_maps, core_ids=list(range(N_CORES)))
    out = np.stack([
        _unblock(res.results[c]["xout"].astype(np.float32)) for c in range(N_CORES)
    ])
    return np.clip(out.reshape(BS, 3, 512, 512), 0.0, 1.0)


if __name__ == "__main__":
    rng = np.random.default_rng(0)
    x = rng.random((BS, 3, 512, 512), dtype=np.float32)
    y = kernel(x, 80)
    print("kernel ran:", y.shape, y.dtype, float(y.min()), float(y.max()))


# revision 10
# speedup vs baseline: 2.8952x; 1.0286x over previous
"""DiffJPEG forward (16x3x512x512, quality=80) on 8 TRN2 NeuronCores.

Data-parallel over batch (2 images/core). The image is pre-blocked on the
host into a [128, 2048] fp16 layout per channel slice where each partition
column holds the 64 pixels of two vertically-adjacent 8x8 blocks. That makes
the full 2D DCT a single dense 128x128 stationary matmul (kron(I2, M (x) M)),
so the whole pipeline is:

  fwd : psum[oc]  = sum_ic  (W2 * 255*WYCC[oc,ic])^T @ x_ic        (9 fp16 matmuls)
  round: q[oc]    = fp16_cast(psum * rq[p] + (1536 + dc_adj[p]))   (ACT; the fp16
         cast at magnitude [1024,2048) has ulp=1, so the cast itself rounds
         t to the nearest integer, RNE — matching jnp.round)
  inv : psum[ro]  = sum_yin (W2 * q[p]*coef/255)^T @ q_yin         (7 fp16 matmuls)
  out : o[ro]     = psum + corr[p]                                  (DVE; corr folds
         the -1536 offset removal and the +128/255 bias)

Host post-processing un-blocks, upcasts and clips to [0,1]. Input is centered
(x - 0.5) on the host to halve fp16 quantization error; the DC shift this
introduces is folded into the round bias. Quant tables replicate the
reference's flattened-(b,c)<16 luma/chroma split, so tables vary per core.
All matmuls run fp16 (1 cyc/row); measured rel-L2 vs the f32 reference is
~6.5e-3, dominated by round()-flips from fp16 input/weight rounding.
"""

import numpy as np

import concourse.bass as bass
import concourse.mybir as mybir
import concourse.tile as tile
from concourse import bacc
from concourse.bass_utils import run_bass_kernel_spmd

N_CORES = 8
BS = 16
IMGS_PER_CORE = BS // N_CORES          # 2
SLICES = IMGS_PER_CORE * 3             # 6
MAGIC = 1536.0                         # fp16 round-to-nearest at ulp=1 for [1024,2048)

F16 = mybir.dt.float16
F32 = mybir.dt.float32
IDENT = mybir.ActivationFunctionType.Identity

_LUM = np.array([[16,11,10,16,24,40,51,61],[12,12,14,19,26,58,60,55],[14,13,16,24,40,57,69,56],[14,17,22,29,51,87,80,62],[18,22,37,56,68,109,103,77],[24,35,55,64,81,104,113,92],[49,64,78,87,103,121,120,101],[72,92,95,98,112,100,103,99]], np.float64)
_CHROM = np.array([[17,18,24,47,99,99,99,99],[18,21,26,66,99,99,99,99],[24,26,56,99,99,99,99,99],[47,66,99,99,99,99,99,99],[99,99,99,99,99,99,99,99],[99,99,99,99,99,99,99,99],[99,99,99,99,99,99,99,99],[99,99,99,99,99,99,99,99]], np.float64)
_WYCC = np.array([[0.299, 0.587, 0.114], [-0.1687, -0.3313, 0.5], [0.5, -0.4187, -0.0813]], np.float64)
_S4TERMS = [
    [(0, 1.0), (2, 1.402)],                       # r
    [(0, 1.0), (1, -0.34414), (2, -0.71414)],     # g
    [(0, 1.0), (1, 1.772)],                       # b
]
_NTERM = 7


def _dct_mat():
    k = np.arange(8)[:, None]
    n = np.arange(8)[None, :]
    norm = np.where(k == 0, np.sqrt(1.0 / 8.0), np.sqrt(2.0 / 8.0))
    return norm * np.cos(np.pi / 8.0 * (n + 0.5) * k)


def _qtables(quality):
    q = max(1, min(100, int(quality)))
    scale = 5000.0 / q if q < 50 else 200.0 - 2.0 * q
    tbs = np.stack([_LUM, _CHROM]) * np.float32(scale)
    return np.clip((tbs.astype(np.float32) + 50.0) / 100.0, 1.0, 255.0).astype(np.float64)


def _w2():
    K64 = np.kron(_dct_mat(), _dct_mat())    # [freq 8u+v, pix 8r+c]
    return np.kron(np.eye(2), K64)           # [128 freq, 128 pix]


def _fwd_weights():
    """fp16 lhsT [pix, freq] per (oc, ic), packed [128, 9*128]."""
    W2 = _w2()
    w = np.zeros((128, 9 * 128), np.float16)
    for oc in range(3):
        for ic in range(3):
            w[:, 128 * (3 * oc + ic):128 * (3 * oc + ic + 1)] = \
                (W2 * (255.0 * _WYCC[oc, ic])).T.astype(np.float16)
    return w


def _core_tables(quality, core, fwdw):
    """Per-core inverse weights + ACT scale/bias + inverse corr vectors."""
    W2 = _w2()
    qt = _qtables(quality)
    invw = np.zeros((128, 2 * _NTERM * 128), np.float16)
    scl = np.zeros((128, SLICES), np.float32)
    bia = np.zeros((128, SLICES), np.float32)
    cor = np.zeros((128, SLICES), np.float32)
    for im in range(IMGS_PER_CORE):
        q2 = []
        rq2 = []
        for ch in range(3):
            tab = qt[0] if (6 * core + 3 * im + ch) < 16 else qt[1]
            qv = np.concatenate([tab.reshape(64), tab.reshape(64)])
            q2.append(qv)
            rq2.append((1.0 / qv).astype(np.float32))
        for oc in range(3):
            scl[:, 3 * im + oc] = rq2[oc]
            # round bias: +MAGIC, Y level shift, and centering compensation
            dct_adj = np.zeros(128, np.float64)
            if oc == 0:
                dct_adj[0] += -1024.0
                dct_adj[64] += -1024.0
            for ic in range(3):
                w = fwdw[:, 128 * (3 * oc + ic):128 * (3 * oc + ic + 1)].astype(np.float64)
                dct_adj += 0.5 * w.sum(axis=0)   # A(0.5*ones) at each freq
            bia[:, 3 * im + oc] = (MAGIC + rq2[oc].astype(np.float64) * dct_adj).astype(np.float32)
        t = 0
        for ro in range(3):
            corr = np.zeros(128, np.float64)
            for (yin, coef) in _S4TERMS[ro]:
                st = (W2 * (q2[yin][:, None] * (coef / 255.0))).astype(np.float16)
                invw[:, 128 * (_NTERM * im + t):128 * (_NTERM * im + t + 1)] = st
                corr += -MAGIC * st.astype(np.float64).sum(axis=0)
                t += 1
            cor[:, 3 * im + ro] = (corr + 128.0 / 255.0).astype(np.float32)
    return invw, scl, bia, cor


def _block(x):
    """[n, 512, 512] f32 -> [n, 128, 2048] blocked fp16 (centered)."""
    n = x.shape[0]
    return np.ascontiguousarray(
        (x - np.float32(0.5)).reshape(n, 32, 2, 8, 64, 8)
        .transpose(0, 2, 3, 5, 1, 4).reshape(n, 128, 2048).astype(np.float16)
    )


def _unblock(y):
    """[n, 128, 2048] -> [n, 512, 512]."""
    n = y.shape[0]
    return y.reshape(n, 2, 8, 8, 32, 64).transpose(0, 4, 1, 2, 5, 3).reshape(n, 512, 512)


def _trace():
    nc = bacc.Bacc("TRN2", target_bir_lowering=False, debug=False)

    xin = nc.dram_tensor("xin", [SLICES, 128, 2048], F16, kind="ExternalInput").ap()
    fwdw_d = nc.dram_tensor("fwdw", [128, 9 * 128], F16, kind="ExternalInput").ap()
    invw_d = nc.dram_tensor("invw", [128, 2 * _NTERM * 128], F16, kind="ExternalInput").ap()
    # vec packs [scl | bia | cor] as [128, 18] f32
    vec_d = nc.dram_tensor("vec", [128, 3 * SLICES], F32, kind="ExternalInput").ap()
    xout = nc.dram_tensor("xout", [SLICES, 128, 2048], F16, kind="ExternalOutput").ap()

    with tile.TileContext(nc) as tc:
        with (
            tc.tile_pool(name="wts", bufs=1) as wp,
            tc.tile_pool(name="xp", bufs=1) as xp,
            tc.tile_pool(name="qp", bufs=1) as qp,
            tc.tile_pool(name="op", bufs=1) as op,
            tc.tile_pool(name="psA", bufs=4, space="PSUM") as psAp,
            tc.tile_pool(name="psB", bufs=4, space="PSUM") as psBp,
        ):
            fwdw = wp.tile([128, 9 * 128], F16, tag="fwdw")
            nc.gpsimd.dma_start(fwdw[:], fwdw_d)

            # PE p-state warmup: burn the ramp on dummy matmuls while the
            # first input chunks are still in flight.
            warm = wp.tile([128, 512], F16, tag="warm")
            nc.vector.memzero(warm[:])
            for _w in range(6):
                wps = psAp.tile([128, 512], F32, tag="psA", name="wps")
                nc.tensor.matmul(wps[:], warm[:, 0:128], warm[:], start=True, stop=True)

            # per-image inputs as one [128, 3*2048] tile; one DMA per
            # 512-column chunk covering all 3 channels (single HWDGE slot)
            xt = [None] * IMGS_PER_CORE
            xt[0] = xp.tile([128, 3 * 2048], F16, tag="x0", name="x0")
            x0v = xt[0][:].rearrange("p (c n) -> p c n", c=3)
            nc.sync.dma_start(
                x0v[:, :, 0:512], xin[0:3].rearrange("c p n -> p c n")[:, :, 0:512]
            )
            vec = wp.tile([128, 3 * SLICES], F32, tag="vec")
            nc.sync.dma_start(vec[:], vec_d)
            for s in range(1, 4):
                nc.sync.dma_start(
                    x0v[:, :, 512 * s:512 * (s + 1)],
                    xin[0:3].rearrange("c p n -> p c n")[:, :, 512 * s:512 * (s + 1)],
                )
            invw = wp.tile([128, 2 * _NTERM * 128], F16, tag="invw")
            nc.gpsimd.dma_start(invw[:], invw_d)
            xt[1] = xp.tile([128, 3 * 2048], F16, tag="x1", name="x1")
            nc.sync.dma_start(
                xt[1][:].rearrange("p (c n) -> p c n", c=3),
                xin[3:6].rearrange("c p n -> p c n"),
            )

            qt_ = [[None] * 3 for _ in range(IMGS_PER_CORE)]
            ot_ = [None] * IMGS_PER_CORE

            def fwd(im, s):
                for oc in range(3):
                    if s == 0 and oc == 0:
                        for c in range(3):
                            qt_[im][c] = qp.tile([128, 2048], F16, tag=f"q{im}_{c}", name=f"q{im}_{c}")
                    ps = psAp.tile([128, 512], F32, tag="psA")
                    for k in range(3):
                        nc.tensor.matmul(
                            ps[:], fwdw[:, 128 * (3 * oc + k):128 * (3 * oc + k + 1)],
                            xt[im][:, 2048 * k + 512 * s:2048 * k + 512 * (s + 1)],
                            start=(k == 0), stop=(k == 2),
                        )
                    sl = 3 * im + oc
                    nc.scalar.activation(
                        qt_[im][oc][:, 512 * s:512 * (s + 1)], ps[:], IDENT,
                        bias=vec[:, SLICES + sl:SLICES + sl + 1],
                        scale=vec[:, sl:sl + 1],
                    )

            def inv(im, s):
                t = 0
                for ro in range(3):
                    if s == 0 and ro == 0:
                        ot_[im] = op.tile([128, 3 * 2048], F16, tag=f"o{im}", name=f"o{im}")
                    terms = _S4TERMS[ro]
                    ps = psBp.tile([128, 512], F32, tag="psB")
                    for ti, (yin, _) in enumerate(terms):
                        nc.tensor.matmul(
                            ps[:], invw[:, 128 * (_NTERM * im + t):128 * (_NTERM * im + t + 1)],
                            qt_[im][yin][:, 512 * s:512 * (s + 1)],
                            start=(ti == 0), stop=(ti == len(terms) - 1),
                        )
                        t += 1
                    sl = 3 * im + ro
                    nc.vector.tensor_scalar_add(
                        ot_[im][:, 2048 * ro + 512 * s:2048 * ro + 512 * (s + 1)],
                        ps[:], vec[:, 2 * SLICES + sl:2 * SLICES + sl + 1],
                    )
                # one DMA for this chunk across all 3 output channels
                nc.sync.dma_start(
                    xout[3 * im:3 * im + 3].rearrange("c p n -> p c n")[:, :, 512 * s:512 * (s + 1)],
                    ot_[im][:].rearrange("p (c n) -> p c n", c=3)[:, :, 512 * s:512 * (s + 1)],
                )

            for im in range(IMGS_PER_CORE):
                fwd(im, 0)
                fwd(im, 1)
                inv(im, 0)
                fwd(im, 2)
                inv(im, 1)
                fwd(im, 3)
                inv(im, 2)
                inv(im, 3)
    nc.compile()
    return nc


_COMPILED = None


def _get_compiled():
    global _COMPILED
    if _COMPILED is None:
        _COMPILED = _trace()
    return _COMPILED


def kernel(img, quality):
    img = np.ascontiguousarray(np.asarray(img, np.float32))
    quality = int(np.asarray(quality))
    nc = _get_compiled()

    fwdw = _fwd_weights()
    in_maps = []
    for core in range(N_CORES):
        invw, scl, bia, cor = _core_tables(quality, core, fwdw)
        shard = img[IMGS_PER_CORE * core:IMGS_PER_CORE * (core + 1)].reshape(SLICES, 512, 512)
        in_maps.append({
            "xin": _block(shard), "fwdw": fwdw, "invw": invw,
            "vec": np.ascontiguousarray(np.concatenate([scl, bia, cor], axis=1)),
        })

    res = run_bass_kernel_spmd(nc, in_maps, core_ids=list(range(N_CORES)))
    out = np.stack([
        _unblock(res.results[c]["xout"].astype(np.float32)) for c in range(N_CORES)
    ])
    return np.clip(out.reshape(BS, 3, 512, 512), 0.0, 1.0)


if __name__ == "__main__":
    rng = np.random.default_rng(0)
    x = rng.random((BS, 3, 512, 512), dtype=np.float32)
    y = kernel(x, 80)
    print("kernel ran:", y.shape, y.dtype, float(y.min()), float(y.max()))
